# revision 1
# baseline (speedup 1.0000x reference)
"""MiniGPT forward pass on 8 Trainium2 NeuronCores (Bass/Tile SPMD kernel), v2.

Model: V=32000, T=2048, D=512, H=8 heads, L=4 layers, DFF=2048, B=2, S=2048.

Sharding (8 cores, one SPMD program):
- Tokens: core c owns 512 tokens = flat[512c : 512c+512] (batch c//4).
- Attention: head-parallel; core c computes head c for both batches over the
  full 2048-token causal window. One merged AllToAll redistributes q,k,v;
  a second AllToAll returns attention outputs to token owners.
- LM head: vocab-parallel in [vocab, token] layout; core c computes
  logits[4000c:4000(c+1), :] for all 4096 tokens after an AllGather of the
  final hidden states. Output DRAM is [4096pad, 4096] bf16; host transposes.

v2 vs v1: host-precomputed broadcast biases, single merged qkv AllToAll,
bn_stats layernorm, causal mask as 0/1 multiply on gpsimd (not PE matmul),
weight-prefetch double buffering, LM head with per-partition bias on the
scalar engine, plain contiguous DMAs and bf16 logits.
"""
import sys

sys.path.insert(0, "/opt/trn_rl_repo")

import numpy as np
import ml_dtypes

import concourse.bass as bass
import concourse.mybir as mybir
import concourse.tile as tile
from concourse import bacc, bass_utils

BF16 = mybir.dt.bfloat16
F32 = mybir.dt.float32
I32 = mybir.dt.int32
AF = mybir.ActivationFunctionType
OP = mybir.AluOpType

V, T, D, H, L = 32000, 2048, 512, 8, 4
HD = D // H          # 64
DFF = 4 * D          # 2048
B, S = 2, 2048
NC = 8               # cores
TOK = 512            # tokens per core
VSH = V // NC        # 4000 vocab rows per core
VSHP = 4096          # padded vocab rows per core


def build_nc():
    nc = bacc.Bacc("TRN2", target_bir_lowering=False, debug=False, num_devices=NC)

    # ---- I/O ----
    h0 = nc.dram_tensor("h0", [TOK, D], F32, kind="ExternalInput")
    wqkvT = nc.dram_tensor("wqkvT", [L, D, 3 * D], BF16, kind="ExternalInput")
    wprojT = nc.dram_tensor("wprojT", [L, D, D], BF16, kind="ExternalInput")
    wffn1T = nc.dram_tensor("wffn1T", [L, D, DFF], BF16, kind="ExternalInput")
    wffn2T = nc.dram_tensor("wffn2T", [L, DFF, D], BF16, kind="ExternalInput")
    bqkv_bc = nc.dram_tensor("bqkv_bc", [128, L * 12], F32, kind="ExternalInput")
    bffn1_bc = nc.dram_tensor("bffn1_bc", [128, L * 16], F32, kind="ExternalInput")
    projb_bc = nc.dram_tensor("projb_bc", [128, L * D], BF16, kind="ExternalInput")
    ffn2b_bc = nc.dram_tensor("ffn2b_bc", [128, L * D], BF16, kind="ExternalInput")
    lmT = nc.dram_tensor("lmT", [D, VSHP], BF16, kind="ExternalInput")
    lmb_col = nc.dram_tensor("lmb_col", [128, 32], F32, kind="ExternalInput")
    mask01 = nc.dram_tensor("mask01", [128, 896], BF16, kind="ExternalInput")
    ident_in = nc.dram_tensor("ident_in", [128, 128], BF16, kind="ExternalInput")
    ones_in = nc.dram_tensor("ones_in", [1, 128], BF16, kind="ExternalInput")
    logits = nc.dram_tensor("logits", [VSHP, B * S], BF16, kind="ExternalOutput")

    # ---- internal DRAM (collective bounces) ----
    qkv_ai = [[nc.dram_tensor(f"qkv_ai{l}_{h}", [3 * D, 256], BF16) for h in (0, 1)]
              for l in range(L)]
    qkv_ao = [[nc.dram_tensor(f"qkv_ao{l}_{h}", [3 * D, 256], BF16) for h in (0, 1)]
              for l in range(L)]
    att_ai = [[nc.dram_tensor(f"att_ai{l}_{h}", [D, 256], BF16) for h in (0, 1)]
              for l in range(L)]
    att_ao = [[nc.dram_tensor(f"att_ao{l}_{h}", [D, 256], BF16) for h in (0, 1)]
              for l in range(L)]
    ag_in = nc.dram_tensor("ag_in", [D, TOK], BF16)
    ag_out = nc.dram_tensor("ag_out", [NC * D, TOK], BF16, addr_space="Shared")
    grp = [list(range(NC))]

    with tile.TileContext(nc) as tc:
        with (
            tc.tile_pool(name="const", bufs=1) as cp,
            tc.tile_pool(name="persist", bufs=1) as pp,
        ):
            ident = cp.tile([128, 128], BF16, name="ident")
            ones_r = cp.tile([1, 128], BF16, name="ones_r")
            msk = cp.tile([128, 896], BF16, name="msk")
            bq_all = cp.tile([128, L * 12], F32, name="bq_all")
            bf1_all = cp.tile([128, L * 16], F32, name="bf1_all")
            pjb = cp.tile([128, L * D], BF16, name="pjb")
            f2b = cp.tile([128, L * D], BF16, name="f2b")
            lmb_sb = cp.tile([128, 32], F32, name="lmb_sb")
            eps_t = cp.tile([128, 1], F32, name="eps_t")
            hts = [pp.tile([128, D], F32, name=f"h{t}") for t in range(4)]
            vones = pp.tile([128, 32 * 65], BF16, name="vones")
            hfT = pp.tile([128, 4 * 512], BF16, name="hfT")

            # ================= prologue =================
            nc.sync.dma_start(out=ident[:], in_=ident_in[:])
            nc.sync.dma_start(out=ones_r[:], in_=ones_in[:])
            nc.sync.dma_start(out=msk[:], in_=mask01[:])
            nc.sync.dma_start(out=bq_all[:], in_=bqkv_bc[:])
            nc.sync.dma_start(out=bf1_all[:], in_=bffn1_bc[:])
            nc.sync.dma_start(out=pjb[:], in_=projb_bc[:])
            nc.sync.dma_start(out=f2b[:], in_=ffn2b_bc[:])
            nc.sync.dma_start(out=lmb_sb[:], in_=lmb_col[:])
            nc.vector.memset(vones[:], 1.0)
            nc.vector.memset(eps_t[:], 1e-5)
            # residual h comes pre-gathered (tok_emb[x] + pos) from the host
            for t in range(4):
                nc.sync.dma_start(out=hts[t][:], in_=h0[128 * t:128 * (t + 1), :])

            with (
                tc.tile_pool(name="wpool", bufs=2) as wp,
                tc.tile_pool(name="work", bufs=2) as wk,
                tc.tile_pool(name="exppool", bufs=4) as ep,
                tc.tile_pool(name="pmm", bufs=2, space="PSUM") as pmm,
                tc.tile_pool(name="psc", bufs=2, space="PSUM") as psc,
                tc.tile_pool(name="pout", bufs=2, space="PSUM") as pout,
            ):
                def load_weights(l):
                    wq = wp.tile([128, 4 * 1536], BF16, tag="wq", name="wq")
                    nc.sync.dma_start(out=wq[:].rearrange("p (c e) -> p c e", c=4),
                                      in_=wqkvT[l].rearrange("(c p) e -> p c e", p=128))
                    wpj = wp.tile([128, 4 * 512], BF16, tag="wpj", name="wpj")
                    nc.sync.dma_start(out=wpj[:].rearrange("p (c e) -> p c e", c=4),
                                      in_=wprojT[l].rearrange("(c p) e -> p c e", p=128))
                    wf1 = wp.tile([128, 4 * 2048], BF16, tag="wf1", name="wf1")
                    nc.sync.dma_start(out=wf1[:].rearrange("p (c e) -> p c e", c=4),
                                      in_=wffn1T[l].rearrange("(c p) e -> p c e", p=128))
                    wf2 = wp.tile([128, 16 * 512], BF16, tag="wf2", name="wf2")
                    nc.sync.dma_start(out=wf2[:].rearrange("p (c e) -> p c e", c=16),
                                      in_=wffn2T[l].rearrange("(c p) e -> p c e", p=128))
                    return wq, wpj, wf1, wf2

                # ---- layernorm split: stats (fused into residual loops) + finish ----
                def ln_stats_tiles():
                    stats = wk.tile([128, 4 * 6], F32, tag="lnstats", name="stats")
                    mv = wk.tile([128, 4 * 2], F32, tag="lnmv", name="mv")
                    return stats, mv

                def ln_stat(stats, mv, srcs, t):
                    nc.vector.bn_stats(out=stats[:, 6 * t:6 * (t + 1)], in_=srcs[t][:])
                    nc.vector.bn_aggr(out=mv[:, 2 * t:2 * (t + 1)], in_=stats[:, 6 * t:6 * (t + 1)])

                def ln_finish(mv, srcs, dst_bf_T, half=None):
                    tiles = {None: (0, 1, 2, 3), 0: (0, 1), 1: (2, 3)}[half]
                    t0 = tiles[0]
                    hln = wk.tile([128, 4 * D], BF16, tag="hln", bufs=1)
                    sd = wk.tile([128, 4], F32, tag="lnsd")
                    rs = wk.tile([128, 4], F32, tag="lnrs")
                    # sd = sqrt(var + eps) on scalar; rs = 1/sd on vector
                    nc.scalar.activation(
                        out=sd[:, t0:t0 + len(tiles)],
                        in_=mv[:].rearrange("p (t two) -> p t two", two=2)[:, t0:t0 + len(tiles), 1],
                        func=AF.Sqrt, bias=eps_t[:])
                    nc.vector.reciprocal(out=rs[:, t0:t0 + len(tiles)], in_=sd[:, t0:t0 + len(tiles)])
                    for t in tiles:
                        nc.vector.tensor_scalar(out=hln[:, D * t:D * (t + 1)], in0=srcs[t][:],
                                                scalar1=mv[:, 2 * t:2 * t + 1], scalar2=rs[:, t:t + 1],
                                                op0=OP.subtract, op1=OP.mult)
                    for f in range(4):
                        tp = pmm.tile([128, 512], BF16, tag="pmm", name="tp")
                        for j, t in enumerate(tiles):
                            nc.tensor.transpose(out=tp[:, 128 * j:128 * (j + 1)],
                                                in_=hln[:, D * t + 128 * f: D * t + 128 * (f + 1)],
                                                identity=ident[:])
                        nc.vector.tensor_copy(
                            out=dst_bf_T[:, 512 * f + 128 * t0:512 * f + 128 * (t0 + len(tiles))],
                            in_=tp[:, 0:128 * len(tiles)])

                # ================= transformer layers =================
                wcur = load_weights(0)
                st1, mv1 = ln_stats_tiles()
                for t in range(4):
                    ln_stat(st1, mv1, hts, t)
                for l in range(L):
                    wq, wpj, wf1, wf2 = wcur

                    # -- LN1 + QKV + A2A per batch half: batch-1 QKV covers
                    # batch-0's collective; batch-0 attention covers batch-1's --
                    hlnT = wk.tile([128, 4 * 512], BF16, tag="hlnT", bufs=1)
                    qkvT = wk.tile([128, 12 * 512], BF16, tag="qkvT", bufs=1)
                    qT = wk.tile([128, 2048], BF16, tag="qT", bufs=1)
                    kT = wk.tile([128, 2048], BF16, tag="kT", bufs=1)
                    vT = wk.tile([128, 2048], BF16, tag="vT", bufs=1)
                    attnT = wk.tile([64, 4096], BF16, tag="attnT", bufs=1)
                    aT = wk.tile([128, 4 * 512], BF16, tag="aT", bufs=1)
                    for h in (0, 1):
                        ln_finish(mv1, hts, hlnT, half=h)
                        for o in [4, 5, 6, 7, 8, 9, 10, 11, 0, 1, 2, 3]:
                            ps = pmm.tile([128, 512], F32, tag="pmm", name="ps")
                            for kc in range(4):
                                nc.tensor.matmul(ps[:, 0:256],
                                                 lhsT=wq[:, 1536 * kc + 128 * o:1536 * kc + 128 * (o + 1)],
                                                 rhs=hlnT[:, 512 * kc + 256 * h:512 * kc + 256 * (h + 1)],
                                                 start=(kc == 0), stop=(kc == 3))
                            nc.scalar.activation(out=qkvT[:, 512 * o + 256 * h:512 * o + 256 * (h + 1)],
                                                 in_=ps[:, 0:256], func=AF.Identity,
                                                 bias=bq_all[:, 12 * l + o:12 * l + o + 1])
                        # shard s rows [192s,192s+192) = head-s k, v, q of my
                        # 256 batch-h tokens (three DMA queues)
                        for s_ in range(8):
                            pb = 64 * (s_ % 2)
                            blk = s_ // 2
                            nc.sync.dma_start(
                                out=qkv_ai[l][h][192 * s_:192 * s_ + 64, :],
                                in_=qkvT[pb:pb + 64, 512 * (4 + blk) + 256 * h:512 * (4 + blk) + 256 * (h + 1)])
                            nc.gpsimd.dma_start(
                                out=qkv_ai[l][h][192 * s_ + 64:192 * s_ + 128, :],
                                in_=qkvT[pb:pb + 64, 512 * (8 + blk) + 256 * h:512 * (8 + blk) + 256 * (h + 1)])
                            nc.scalar.dma_start(
                                out=qkv_ai[l][h][192 * s_ + 128:192 * s_ + 192, :],
                                in_=qkvT[pb:pb + 64, 512 * blk + 256 * h:512 * blk + 256 * (h + 1)])
                        nc.gpsimd.collective_compute(
                            "AllToAll", OP.bypass, replica_groups=grp,
                            ins=[qkv_ai[l][h][:]], outs=[qkv_ao[l][h][:]],
                        )
                        if h == 0:
                            # prefetch next layer's weights during collective 0
                            if l + 1 < L:
                                wcur = load_weights(l + 1)
                        else:
                            # bias pre-add only after batch-1's LN apply issued
                            for t in range(4):
                                nc.vector.tensor_tensor(out=hts[t][:], in0=hts[t][:],
                                                        in1=pjb[:, D * l:D * (l + 1)], op=OP.add)

                    # -- per batch: receive kqv, transpose v, attention, A2A out --
                    for b_ in range(2):
                        hb = 64 * b_
                        for r in range(8):
                            nc.sync.dma_start(out=kT[hb:hb + 64, 256 * r:256 * (r + 1)],
                                              in_=qkv_ao[l][b_][192 * r:192 * r + 64, :])
                            nc.gpsimd.dma_start(out=vT[hb:hb + 64, 256 * r:256 * (r + 1)],
                                                in_=qkv_ao[l][b_][192 * r + 64:192 * r + 128, :])
                            nc.scalar.dma_start(out=qT[hb:hb + 64, 256 * r:256 * (r + 1)],
                                                in_=qkv_ao[l][b_][192 * r + 128:192 * r + 192, :])
                        for i in range(16):
                            tp = pmm.tile([128, 64], BF16, tag="pmm", name="tp2")
                            nc.tensor.transpose(out=tp[:], in_=vT[hb:hb + 64, 128 * i:128 * (i + 1)],
                                                identity=ident[hb:hb + 64, hb:hb + 64])
                            nc.vector.tensor_copy(out=vones[:, 65 * (16 * b_ + i):65 * (16 * b_ + i) + 64],
                                                  in_=tp[:])
                        for p in range(2):
                            outp = pout.tile([65, 1024], F32, tag="pout", name="outp")
                            for i in range(8 * p + 8):
                                jlmin = max(0, i // 4 - 2 * p)
                                for jl in (0, 1):
                                    if jl < jlmin:
                                        continue
                                    diag = (i // 4 == 2 * p + jl)
                                    qb = 1024 * p + 512 * jl
                                    # skip query cols that are fully masked for
                                    # this key block (diag blocks only)
                                    ow = 128 * (i % 4) if diag else 0
                                    sc = psc.tile([128, 512], F32, tag="psc", name="sc")
                                    nc.tensor.matmul(
                                        sc[:, ow:],
                                        lhsT=kT[hb:hb + 64, 128 * i:128 * (i + 1)],
                                        rhs=qT[hb:hb + 64, qb + ow:qb + 512],
                                        start=True, stop=True)
                                    ex = ep.tile([128, 512], BF16, tag="ex")
                                    nc.scalar.activation(out=ex[:, ow:], in_=sc[:, ow:],
                                                         func=AF.Exp, scale=float(HD) ** -0.5)
                                    if diag:
                                        nc.vector.tensor_tensor(
                                            out=ex[:, ow:ow + 128], in0=ex[:, ow:ow + 128],
                                            in1=msk[:, 384:512], op=OP.mult)
                                    kmax = 4 * (2 * p + jl) + 3
                                    nc.tensor.matmul(
                                        outp[:, 512 * jl + ow:512 * (jl + 1)],
                                        lhsT=vones[:, 65 * (16 * b_ + i):65 * (16 * b_ + i + 1)],
                                        rhs=ex[:, ow:],
                                        start=(i == 0), stop=(i == kmax))
                            # normalize: rows 0..63 /= row 64
                            dnb = wk.tile([1, 1024], BF16, tag="rcb", bufs=1)
                            nc.vector.tensor_copy(out=dnb[:], in_=outp[64:65, :])
                            for q2 in range(2):
                                bc = psc.tile([64, 512], F32, tag="psc", name="bc")
                                nc.tensor.matmul(bc[:], lhsT=ones_r[:, 0:64],
                                                 rhs=dnb[:, 512 * q2:512 * (q2 + 1)], start=True, stop=True)
                                rcs = wk.tile([64, 512], F32, tag="bcs", bufs=1)
                                nc.vector.reciprocal_approx_fast(out=rcs[:], in_=bc[:])
                                nc.vector.tensor_tensor(
                                    out=attnT[:, 2048 * b_ + 1024 * p + 512 * q2:2048 * b_ + 1024 * p + 512 * (q2 + 1)],
                                    in0=outp[0:64, 512 * q2:512 * (q2 + 1)], in1=rcs[:], op=OP.mult)
                        # A2A this batch's attention outputs back to owners;
                        # overlapped by the other batch's attention / proj
                        for s_ in range(8):
                            q_ = (nc.sync, nc.gpsimd, nc.scalar)[s_ % 3]
                            q_.dma_start(out=att_ai[l][b_][64 * s_:64 * (s_ + 1), :],
                                         in_=attnT[:, 2048 * b_ + 256 * s_:2048 * b_ + 256 * (s_ + 1)])
                        nc.gpsimd.collective_compute(
                            "AllToAll", OP.bypass, replica_groups=grp,
                            ins=[att_ai[l][b_][:]], outs=[att_ao[l][b_][:]],
                        )
                        for fc in range(4):
                            nc.sync.dma_start(
                                out=aT[:, 512 * fc + 256 * b_:512 * fc + 256 * (b_ + 1)],
                                in_=att_ao[l][b_][128 * fc:128 * (fc + 1), :])

                    # -- proj + LN2 + FFN per batch half: batch-0 FFN covers
                    # batch-1's attention-output collective --
                    st2, mv2 = ln_stats_tiles()
                    hln2T = wk.tile([128, 4 * 512], BF16, tag="hlnT", bufs=1)
                    fT = wk.tile([128, 16 * 512], BF16, tag="fT", bufs=1)
                    st1, mv1 = ln_stats_tiles()
                    for hh in (0, 1):
                        for t in (2 * hh, 2 * hh + 1):
                            ps = pmm.tile([128, 512], F32, tag="pmm", name="ps2")
                            for fc in range(4):
                                nc.tensor.matmul(ps[:],
                                                 lhsT=aT[:, 512 * fc + 128 * t:512 * fc + 128 * (t + 1)],
                                                 rhs=wpj[:, 512 * fc:512 * (fc + 1)],
                                                 start=(fc == 0), stop=(fc == 3))
                            nc.vector.tensor_tensor(out=hts[t][:], in0=hts[t][:], in1=ps[:], op=OP.add)
                            ln_stat(st2, mv2, hts, t)
                        ln_finish(mv2, hts, hln2T, half=hh)
                        # ffn2-bias pre-add (after this half's LN2 apply)
                        for t in (2 * hh, 2 * hh + 1):
                            nc.vector.tensor_tensor(out=hts[t][:], in0=hts[t][:],
                                                    in1=f2b[:, D * l:D * (l + 1)], op=OP.add)
                        for o in range(16):
                            ps = pmm.tile([128, 512], F32, tag="pmm", name="ps3")
                            for kc in range(4):
                                nc.tensor.matmul(ps[:, 0:256],
                                                 lhsT=wf1[:, 2048 * kc + 128 * o:2048 * kc + 128 * (o + 1)],
                                                 rhs=hln2T[:, 512 * kc + 256 * hh:512 * kc + 256 * (hh + 1)],
                                                 start=(kc == 0), stop=(kc == 3))
                            nc.scalar.activation(out=fT[:, 512 * o + 256 * hh:512 * o + 256 * (hh + 1)],
                                                 in_=ps[:, 0:256],
                                                 func=AF.Gelu, bias=bf1_all[:, 16 * l + o:16 * l + o + 1])
                        for t in (2 * hh, 2 * hh + 1):
                            ps = pmm.tile([128, 512], F32, tag="pmm", name="ps4")
                            for kc in range(16):
                                nc.tensor.matmul(ps[:],
                                                 lhsT=fT[:, 512 * kc + 128 * t:512 * kc + 128 * (t + 1)],
                                                 rhs=wf2[:, 512 * kc:512 * (kc + 1)],
                                                 start=(kc == 0), stop=(kc == 15))
                            nc.vector.tensor_tensor(out=hts[t][:], in0=hts[t][:], in1=ps[:], op=OP.add)
                            ln_stat(st1, mv1, hts, t)

                # ================= final LN + AllGather =================
                ln_finish(mv1, hts, hfT)
                for f in range(4):
                    nc.sync.dma_start(out=ag_in[128 * f:128 * (f + 1), :],
                                      in_=hfT[:, 512 * f:512 * (f + 1)])
                nc.gpsimd.collective_compute(
                    "AllGather", OP.bypass, replica_groups=[list(range(NC))],
                    ins=[ag_in[:]], outs=[ag_out[:]],
                )

            # ================= LM head (vocab-major, body pools closed) ======
            with (
                tc.tile_pool(name="lmwpool", bufs=1) as lwp,
                tc.tile_pool(name="lmwork", bufs=3) as lk,
                tc.tile_pool(name="lmstage", bufs=6) as ls,
                tc.tile_pool(name="plm", bufs=4, space="PSUM") as plm,
            ):
                # 4MB LM weight load; overlaps the AllGather wait
                lmw = lwp.tile([128, 4 * VSHP], BF16, name="lmw")
                nc.sync.dma_start(out=lmw[:].rearrange("p (c e) -> p c e", c=4)[:, 0:2],
                                  in_=lmT[:].rearrange("(c p) e -> p c e", p=128)[:, 0:2])
                nc.scalar.dma_start(out=lmw[:].rearrange("p (c e) -> p c e", c=4)[:, 2:4],
                                    in_=lmT[:].rearrange("(c p) e -> p c e", p=128)[:, 2:4])

                def lm_tile(r, rhs_slices):
                    # logits block [vocab 128v.., tok 512r..] for all 32 v-chunks
                    for v in range(32):
                        ps = plm.tile([128, 512], F32, tag="plm", name="lps")
                        for kc in range(4):
                            nc.tensor.matmul(
                                ps[:],
                                lhsT=lmw[:, VSHP * kc + 128 * v:VSHP * kc + 128 * (v + 1)],
                                rhs=rhs_slices[kc],
                                start=(kc == 0), stop=(kc == 3))
                        st = ls.tile([128, 512], BF16, tag="st", name="lst")
                        nc.scalar.activation(out=st[:], in_=ps[:], func=AF.Identity,
                                             bias=lmb_sb[:, v:v + 1])
                        nc.sync.dma_start(out=logits[128 * v:128 * (v + 1), 512 * r:512 * (r + 1)],
                                          in_=st[:])

                # all 8 token tiles come from ag_out (own rows included)
                for r in range(8):
                    lhs = lk.tile([128, 2048], BF16, tag="lmlhs", name="lmlhs")
                    for kc in range(4):
                        nc.sync.dma_start(
                            out=lhs[:, 512 * kc:512 * (kc + 1)],
                            in_=ag_out[:].rearrange("(r k p) t -> r k p t", r=8, p=128)[r, kc])
                    lm_tile(r, [lhs[:, 512 * kc:512 * (kc + 1)] for kc in range(4)])

    nc.compile()
    return nc


_NC_CACHE = None


def _get_nc():
    global _NC_CACHE
    if _NC_CACHE is None:
        _NC_CACHE = build_nc()
    return _NC_CACHE


def _prep_inputs(inputs):
    bf = ml_dtypes.bfloat16
    tok_emb = np.asarray(inputs["tok_emb"], np.float32)
    pos_emb = np.asarray(inputs["pos_emb"], np.float32)
    x = np.asarray(inputs["x"]).astype(np.int32).reshape(-1)  # [4096] flat

    def eff(w, g, b, wb):
        # fold the preceding layernorm's gamma/beta into w (out,in) and bias
        w = np.asarray(w, np.float32)
        weff = w * np.asarray(g, np.float32)[None, :]
        beff = w @ np.asarray(b, np.float32) + np.asarray(wb, np.float32)
        return weff, beff

    wqkvT = np.zeros((L, D, 3 * D), bf)
    bqkv = np.zeros((L, 12, 128), np.float32)
    wprojT = np.zeros((L, D, D), bf)
    bproj = np.zeros((L, D), np.float32)
    wffn1T = np.zeros((L, D, DFF), bf)
    bffn1 = np.zeros((L, 16, 128), np.float32)
    wffn2T = np.zeros((L, DFF, D), bf)
    bffn2 = np.zeros((L, D), np.float32)
    for l in range(L):
        w, b = eff(inputs["qkv_w"][l], inputs["ln1_g"][l], inputs["ln1_b"][l], inputs["qkv_b"][l])
        wqkvT[l] = w.T.astype(bf)
        bqkv[l] = b.reshape(12, 128)
        wprojT[l] = np.asarray(inputs["proj_w"][l], np.float32).T.astype(bf)
        bproj[l] = np.asarray(inputs["proj_b"][l], np.float32)
        w, b = eff(inputs["ffn1_w"][l], inputs["ln2_g"][l], inputs["ln2_b"][l], inputs["ffn1_b"][l])
        wffn1T[l] = w.T.astype(bf)
        bffn1[l] = b.reshape(16, 128)
        wffn2T[l] = np.asarray(inputs["ffn2_w"][l], np.float32).T.astype(bf)
        bffn2[l] = np.asarray(inputs["ffn2_b"][l], np.float32)
    lmw, lmbf = eff(inputs["lm_w"], inputs["lnf_g"], inputs["lnf_b"], inputs["lm_b"])

    # bias broadcast tables (same 128 rows)
    bqkv_bc = np.ascontiguousarray(bqkv.transpose(2, 0, 1).reshape(128, L * 12))
    bffn1_bc = np.ascontiguousarray(bffn1.transpose(2, 0, 1).reshape(128, L * 16))
    projb_bc = np.broadcast_to(bproj.reshape(1, L * D), (128, L * D)).astype(bf)
    ffn2b_bc = np.broadcast_to(bffn2.reshape(1, L * D), (128, L * D)).astype(bf)

    # causal 0/1 strip: msk[kk, cc] = 1 where kk <= cc - 384
    kk = np.arange(128)[:, None]
    cc = np.arange(896)[None, :]
    mask = (kk <= cc - 384).astype(np.float32).astype(bf)

    common = dict(wqkvT=wqkvT, wprojT=wprojT, wffn1T=wffn1T,
                  wffn2T=wffn2T, bqkv_bc=bqkv_bc, bffn1_bc=bffn1_bc,
                  projb_bc=projb_bc, ffn2b_bc=ffn2b_bc, mask01=mask,
                  ident_in=np.eye(128, dtype=bf), ones_in=np.ones((1, 128), bf))
    in_maps = []
    for c in range(NC):
        s0 = 512 * (c % 4)
        m = dict(common)
        # tiles 0,1 = my 256 batch-0 tokens; tiles 2,3 = my 256 batch-1
        # tokens (same positions 256c..256c+256 in each batch)
        p0 = 256 * c
        pe = pos_emb[p0:p0 + 256]
        m["h0"] = np.concatenate([
            tok_emb[x[p0:p0 + 256]] + pe,
            tok_emb[x[2048 + p0:2048 + p0 + 256]] + pe,
        ], axis=0)
        lmw_pad = np.zeros((VSHP, D), np.float32)
        lmw_pad[:VSH] = lmw[VSH * c:VSH * (c + 1)]
        lmb_pad = np.zeros((VSHP,), np.float32)
        lmb_pad[:VSH] = lmbf[VSH * c:VSH * (c + 1)]
        m["lmT"] = np.ascontiguousarray(lmw_pad.T.astype(bf))
        m["lmb_col"] = np.ascontiguousarray(lmb_pad.reshape(32, 128).T)
        in_maps.append(m)
    return in_maps


def run(inputs, trace=False, tmpdir=None):
    nc = _get_nc()
    in_maps = _prep_inputs(inputs)
    res = bass_utils.run_bass_kernel_spmd(nc, in_maps, list(range(NC)), trace=trace, tmpdir=tmpdir)
    full = np.empty((B * S, V), np.float32)
    # logits col 512r+j is core r's local token j: batch 0 tokens first
    perm = np.empty(B * S, np.int64)
    for r in range(NC):
        perm[512 * r:512 * r + 256] = 256 * r + np.arange(256)
        perm[512 * r + 256:512 * (r + 1)] = 2048 + 256 * r + np.arange(256)
    for c in range(NC):
        full[perm, VSH * c:VSH * (c + 1)] = \
            np.asarray(res.results[c]["logits"][:VSH], np.float32).T
    return full.reshape(B, S, V), res


def kernel(**inputs) -> np.ndarray:
    out, _ = run(inputs)
    return out

